# revision 38
# baseline (speedup 1.0000x reference)
"""BlurPool3d (depthwise [1,2,1]^3/64 blur, stride 2, replicate pad) on 8 Trainium2 cores.

Input  x: (4, 64, 32, 112, 112) fp32  ->  out: (4, 64, 16, 56, 56) fp32.

Strategy
--------
256 independent (n, c) slices of (32, 112, 112) -> (16, 56, 56); pure data
parallel over 8 cores (32 slices/core), tiles of 8 slices on
[128 partitions = (d-pair db 16, slice s 8)], h chunked by 14 output rows.
The kernel is DMA-bound (51.4 MB of fp32 input per core); everything else
is arranged so the serial DMA resource streams loads back to back.

Engine split (per chunk; W/H stages emitted per di so each half chains off
its own load DMA, halving pipeline latency at ramp and drain):
  * SP   : two HBM loads X[:, di] = [128, 28, 112] f32, rows 2j0..2j0+27
           (no overlap between chunks; the only traffic on the SP queue).
  * Act  : deinterleave-cast to fp16 with the blur scales folded in:
           Xe2 = (2/64)x[even w], Xo = (1/64)x[odd w], Xl3 = (3/64)x[0]
           (scaling is free in the activation op, and /64 up front means the
           last D-stage adds produce final output values directly).
  * DVE  : all three convs as fp16 tensor_add only (fp16 unit-stride
           tensor_add gets the 2x packed DVE mode; scalar_tensor_tensor
           never gets a fast mode, so the 1-2-1 taps are decomposed into
           pairwise adds):
             W: u = Xe2 + Xo + shift(Xo); w=0 edge via Xl3; the u(2j0-1)
                boundary row is carried from the previous chunk's U tile
                (replicated for chunk 0) instead of re-loading the row.
             H: p[r] = u[r]+u[r+1];  v[j] = p[2j]+p[2j+1].
             D: y = (v0+v1) + v0 written into the fp16 out tile, plus
                y[0:8] += v0 (the d-replicate edge; an engine AP may start
                at partition>0 only if it spans <=32 partitions, so base 0
                span 8 is legal).
  * Pool : one SWDGE accum-DMA per chunk adds the cross-partition d-term
           y[db] += v1[db-1] (partition shift by 8; engines cannot shift
           partitions, DMA can).
  * Act  : per-chunk fp16 output store, deferred two chunks so it never
           stalls the deint stream on the shift DMA; host upcasts to f32
           (rel err ~1.2e-3 vs the 2e-2 gate).

NOTE: TensorE is useless in this environment (~65us fixed cost per matmul
instruction on the axon-virtualized NeuronCores).
"""

import numpy as np

import concourse.bass as bass
import concourse.tile as tile
from concourse import mybir
from concourse.bass_utils import run_bass_kernel_spmd
from concourse.vector_clock import ScopedClock, VectorClock

# ---------------------------------------------------------------------------
# Workaround: this container's walrus (nix b16 neuronxcc) rejects ANY
# instruction carrying >1 sync wait ("Too many sync wait commands",
# CoreV2/V3GenImpl setupSyncWait).  Tile's kernel-tail drain and many
# scheduled instructions carry several.  Split those waits across nofuse
# NOPs (1 wait each) on the same engine, inserted immediately before.
_MAX_TAIL_WAITS = 1


def _split_drain_and_barrier(self, tick_clock, wait_clock):
    gc = tick_clock.global_clock
    n = len(gc)
    procs = [p for p in range(n) if gc[p] > 0]
    for i in range(0, len(procs), _MAX_TAIL_WAITS):
        chunk = set(procs[i : i + _MAX_TAIL_WAITS])
        sub = VectorClock([gc[p] if p in chunk else 0 for p in range(n)])
        nop = self.nc.sync.nop(nofuse=True)
        wait_clock.add_sem_waits(nop.ins, ScopedClock({None: sub}))
    # The NOPs above already hold the SP queue until every sem fires; the
    # drain needs no waits of its own (SP executes its stream in order).
    self.nc.sync.drain()
    self.nc.all_engine_barrier()
    assert self.sems is not None
    popped = self.nc._tile_sem_poison_stack.pop()
    assert popped is self._sem_poison
    self.nc.clear_and_free_semaphores(list(self.sems.allocated().values()))
    self.nc.all_engine_barrier()


tile.TileContext._drain_and_barrier = _split_drain_and_barrier


_ORIG_LOWER = tile.TileContext._lower_ordered_insts


def _split_waits_and_lower(self, ordered):
    """Hoist all-but-one sync wait of every scheduled instruction onto
    single-wait NOPs on the same engine, immediately before it."""
    nc = self.nc
    for bb_name, insts in ordered.items():
        new = []
        for inst in insts:
            si = getattr(inst, "sync_info", None)
            cls = type(inst).__name__
            if (
                si is not None
                and len(si.on_wait) > 1
                and not cls.startswith("BassTile")
                and not cls.startswith("Tile")
            ):
                waits = list(si.on_wait)
                for w in waits[:-1]:
                    nop = mybir.InstNoOp(
                        name=nc.get_next_instruction_name(),
                        engine=inst.engine,
                        bass_nofuse=True,
                        sync_info=mybir.SyncInfo(on_wait=[w], on_update=[]),
                    )
                    new.append(nop)
                inst.sync_info = mybir.SyncInfo(
                    on_wait=[waits[-1]], on_update=list(si.on_update)
                )
            new.append(inst)
        ordered[bb_name] = new
    return _ORIG_LOWER(self, ordered)


tile.TileContext._lower_ordered_insts = _split_waits_and_lower
# ---------------------------------------------------------------------------

N_CORES = 8
NB, CH = 4, 64
D, H, W = 32, 112, 112
DO, HO, WO = 16, 56, 56
SLICES = NB * CH              # 256
SPC = SLICES // N_CORES       # 32 slices per core
TS = 8                        # slices per tile: partitions = (db 16, s 8)
NCH = 14                      # output h rows per chunk
NR = 2 * NCH + 1              # 29 input rows per chunk

F32 = mybir.dt.float32
F16 = mybir.dt.float16
_ADD = mybir.AluOpType.add


def build_nc(
    n_slices: int = SPC,
    repeat: int = 1,
    xbufs: int = 2,
    chunk_plans=None,
) -> bass.Bass:
    """Per-core Bass program (see module docstring).

    chunk_plans: (first, mid, last) chunk-size lists (each summing to
    HO=56) applied to the first / middle / last tile of the sequence,
    shaping pipeline ramp and drain.
    """
    if chunk_plans is None:
        mid = [NCH] * (HO // NCH)
        chunk_plans = (mid, mid, mid)
    for plan in chunk_plans:
        assert sum(plan) == HO, plan
    first_plan, mid_plan, last_plan = chunk_plans
    ntiles = n_slices // TS
    assert n_slices % TS == 0
    nc = bass.Bass("TRN2", target_bir_lowering=False, debug=False, enable_asserts=False)
    x_d = nc.dram_tensor("x", [n_slices, D, H, W], F32, kind="ExternalInput").ap()
    y_d = nc.dram_tensor("y", [n_slices, DO, HO, WO], F16, kind="ExternalOutput").ap()

    with tile.TileContext(nc) as tc:
        with (
            tc.tile_pool(name="xin", bufs=xbufs) as xp,
            tc.tile_pool(name="deint", bufs=2) as dp,
            tc.tile_pool(name="ubuf", bufs=3) as up,
            tc.tile_pool(name="pbuf", bufs=2) as pp,
            tc.tile_pool(name="vbuf", bufs=2) as vp,
            tc.tile_pool(name="tbuf", bufs=2) as tp,
            tc.tile_pool(name="ybuf", bufs=2) as yp,
        ):
            pending_stores = []

            def flush_store(keep: int = 0):
                while len(pending_stores) > keep:
                    dst, src = pending_stores.pop(0)
                    nc.scalar.dma_start(dst, src)

            tiles = [i for _ in range(repeat) for i in range(ntiles)]
            for ti, it in enumerate(tiles):
                s0 = it * TS
                x_v = x_d[s0 : s0 + TS].rearrange("s (db di) h w -> db s di h w", di=2)
                Yf = yp.tile([128, HO, WO], F16, name="Yf", tag="Yf")
                plan = (
                    first_plan
                    if ti == 0
                    else (last_plan if ti == len(tiles) - 1 else mid_plan)
                )
                j0 = 0
                Uprev, prev_nr = None, None
                for c, nch in enumerate(plan):
                    nr = 2 * nch + 1
                    nx = 2 * nch  # x rows loaded; u row -1 is carried, not reloaded
                    # ---- load: X rows = x rows 2*j0 .. 2*j0+2*nch-1
                    X = xp.tile([128, 2, nx, W], F32, name="X", tag=f"X{nch}")
                    for di in range(2):
                        nc.sync.dma_start(
                            X[:, di, :, :],
                            x_v[:, :, di, 2 * j0 : 2 * j0 + nx, :].rearrange(
                                "db s r w -> db s (r w)"
                            ),
                        )

                    # ---- per di: Act deint-cast to fp16, DVE W-conv, DVE
                    # H-conv (di-split so each half chains off its own load
                    # DMA -- halves the pipeline latency at ramp and drain)
                    # (the blur's /64 is folded into the cast scales, so the
                    # last D-stage adds produce the final output directly)
                    Xe2 = dp.tile([128, 2, nx, WO], F16, name="Xe2", tag=f"Xe2{nch}")
                    Xo = dp.tile([128, 2, nx, WO], F16, name="Xo", tag=f"Xo{nch}")
                    Xl3 = dp.tile([128, 2, nx, 1], F16, name="Xl3", tag=f"Xl3{nch}")
                    U = up.tile([128, 2, nr, WO], F16, name="U", tag=f"U{nch}")
                    P = pp.tile([128, 2, nr - 1, WO], F16, name="P", tag=f"P{nch}")
                    V = vp.tile([128, 2, nch, WO], F16, name="V", tag=f"V{nch}")
                    for di in range(2):
                        # Act: deinterleave x2-folded cast of this di's rows
                        Xs = X[:, di]                       # [128, nx, 112] f32
                        nc.scalar.mul(Xe2[:, di], Xs[:, :, 0 : 2 * WO : 2], 2.0 / 64.0)
                        nc.scalar.mul(Xo[:, di], Xs[:, :, 1 : 2 * WO : 2], 1.0 / 64.0)
                        nc.scalar.mul(Xl3[:, di], Xs[:, :, 0:1], 3.0 / 64.0)  # w edge
                        # DVE W-conv: u[w'] = x[2w'-1] + 2x[2w'] + x[2w'+1]
                        # into U rows 1..nr-1; U row 0 = u(2*j0-1) carried
                        # from the previous chunk (replicated for chunk 0)
                        Us = U[:, di, 1:nr, :]
                        nc.vector.tensor_add(Us, Xe2[:, di], Xo[:, di])
                        nc.vector.tensor_add(
                            Us[:, :, 1:WO], Us[:, :, 1:WO], Xo[:, di, :, 0 : WO - 1]
                        )
                        nc.vector.tensor_add(Us[:, :, 0:1], Xl3[:, di], Xo[:, di, :, 0:1])
                        if c == 0:
                            nc.vector.tensor_copy(U[:, di, 0:1, :], U[:, di, 1:2, :])
                        else:
                            nc.vector.tensor_copy(
                                U[:, di, 0:1, :], Uprev[:, di, prev_nr - 1 : prev_nr, :]
                            )
                        # DVE H-conv: p[r] = u[r]+u[r+1]; v[j] = p[2j]+p[2j+1]
                        nc.vector.tensor_add(
                            P[:, di], U[:, di, 0 : nr - 1, :], U[:, di, 1:nr, :]
                        )
                        nc.vector.tensor_add(
                            V[:, di],
                            P[:, di, 0 : nr - 2 : 2, :],
                            P[:, di, 1 : nr - 1 : 2, :],
                        )
                    Uprev, prev_nr = U, nr
                    flush_store(keep=1)  # out rows from two chunks back

                    # ---- DVE D-conv (partition-local): y = 2*v0 + v1,
                    # written straight into the output tile rows; db==0
                    # replicate (v[-1] = v0) is one extra 8-partition TT
                    Q = tp.tile([128, nch, WO], F16, name="Q", tag=f"Q{nch}")
                    Yc = Yf[:, j0 : j0 + nch, :]
                    nc.vector.tensor_add(Q[:], V[:, 0], V[:, 1])
                    nc.vector.tensor_add(Yc, Q[:], V[:, 0])
                    nc.vector.tensor_add(Yc[0:TS], Yc[0:TS], V[0:TS, 0])
                    # cross-partition term via SWDGE accum-DMA:
                    # y[db] += v1[db-1] (partition shift by 8)
                    nc.gpsimd.dma_start(Yc[TS:128], V[0 : 128 - TS, 1], accum_op=_ADD)

                    pending_stores.append((
                        y_d[s0 : s0 + TS, :, j0 : j0 + nch, :].rearrange(
                            "s d h w -> d s (h w)"
                        ),
                        Yc.rearrange("p h w -> p (h w)"),
                    ))
                    j0 += nch

            flush_store()
    return nc


_CACHED_NC = {}


def _get_nc(repeat: int = 1):
    if repeat not in _CACHED_NC:
        _CACHED_NC[repeat] = build_nc(repeat=repeat)
    return _CACHED_NC[repeat]


def run(x: np.ndarray, trace: bool = False, repeat: int = 1, **kw):
    """Shard, run on 8 cores, gather. Returns (y_full, BassKernelResults)."""
    x = np.ascontiguousarray(np.asarray(x, dtype=np.float32))
    assert x.shape == (NB, CH, D, H, W), x.shape
    xr = x.reshape(SLICES, D, H, W)
    in_maps = [
        {"x": np.ascontiguousarray(xr[k * SPC : (k + 1) * SPC])}
        for k in range(N_CORES)
    ]
    res = run_bass_kernel_spmd(
        _get_nc(repeat), in_maps, list(range(N_CORES)), trace=trace, **kw
    )
    y = np.concatenate([res.results[k]["y"] for k in range(N_CORES)], axis=0)
    return y.reshape(NB, CH, DO, HO, WO).astype(np.float32), res


def kernel(x: np.ndarray) -> np.ndarray:
    y, _ = run(x)
    return y
